# revision 28
# baseline (speedup 1.0000x reference)
"""Trainium2 Bass kernel for the raw-reshape RoPE attention problem.

Math structure (verified against the reference):
  The reference reshapes [B, N, H*D] -> [B, H, N, D] with a *raw* reshape
  (no transpose): head h only sees input tokens [h*128, (h+1)*128) and the
  output rows [h*128, (h+1)*128) depend only on head h.  The 32 (b, h)
  pairs are fully independent: 4 pairs per NeuronCore, no collectives.
  Within a head, rows are permuted j' = s*128 + t (s = weight block,
  t = token); softmax/attention are invariant to that permutation.  RoPE
  even/odd channels are pre-permuted on the host so the rotation is
  contiguous-block arithmetic (cancels inside q.k).

SINGLE-TILING-MODE DESIGN.  Switching the PE between tiling modes (e.g.
(64,128) row-tiled vs (128,128) full) drains the array at ~250ns per
switch, so EVERY matmul in this kernel is shaped (row=64, col=128) and
runs on PE tiles (0,0)/(64,0), which execute CONCURRENTLY:

  scores   lhsT = k-chunk channels [64, 128keys]; tile T0 computes the
           even chunk while T8 computes the odd chunk (2x the effective
           rate of the K=64-contraction scores).  q channels duplicated
           across both partition halves (SBUF->SBUF DMA).
  AV       contraction (128 keys) split: T0 takes keys 0:63 -> accA,
           T8 keys 64:127 -> accB (different PSUM banks as required for
           row tiles); normalize adds accA+accB on DVE.
  proj     contraction (128 chans per k-chunk) split the same way:
           T0 -> ppA, T8 -> ppB; the A+B add folds into rope (q/k), the
           vsb copy (v), and the osb copy (wo) as DVE tensor_adds.
  transp   out = x.T via row-tiled matmuls against identity blocks
           ident[0:64,0:64] / ident[64:128,64:128] (token halves ->
           separate PSUM banks), NOT transpose-mode (which would be a
           mode switch).

  E = exp(s/8) on ScalarE over flat [128, 1024] PSUM (two chunks: cols
  0:512 even, 512:1024 odd).  Softmax sums ride column 0 of vsb
  ([1 | zeros(63) | V]) into acc partition 0.  acc is QUARTER-sized
  ([128, 4s, 128t]; 4 quarters per pair) so accA+accB fit in 2 PSUM
  banks: psS 2x2 + psA 2 + psW 2 = 8 banks.

Schedule: pair-level software pipeline as before -- proj/rope/transposes
of pair p+1 and the deferred wo-projection of pair p-1 are emitted inside
pair p's attention, landing in the PE slack of the ScalarE-bound exp
stream.
"""

import numpy as np
import ml_dtypes

N_CORES = 8
B, N, DIM = 2, 2048, 1024
H, HD = 16, 64
PAIRS_PER_CORE = 4
BF16 = ml_dtypes.bfloat16

_CACHE = {}


def _rope_tables():
    """cos/sin tables [128 t, 16 s, 32 i]; (t=0, s=0) is the unrotated row."""
    inv = 1.0 / (10000.0 ** (np.arange(0, HD, 2, dtype=np.float32) / HD))
    pos = np.arange(128 * 16, dtype=np.float32).reshape(128, 16) - 1.0  # j-1
    ang = pos[:, :, None] * inv[None, None, :]          # [128, 16, 32]
    c = np.cos(ang).astype(np.float32)
    s = np.sin(ang).astype(np.float32)
    c[0, 0, :] = 1.0
    s[0, 0, :] = 0.0
    return c, s


def _chan_perm():
    """c = s*64 + 2i + par -> c' = s*64 + par*32 + i."""
    perm = np.zeros(DIM, np.int64)
    for s in range(16):
        i = np.arange(32)
        perm[s * 64 + i] = s * 64 + 2 * i
        perm[s * 64 + 32 + i] = s * 64 + 2 * i + 1
    return perm


def _build_nc():
    import concourse.mybir as mybir
    import concourse.tile as tile
    from concourse import bacc
    from concourse.masks import make_identity

    dt = mybir.dt
    AF = mybir.ActivationFunctionType

    nc = bacc.Bacc("TRN2", target_bir_lowering=False, debug=False,
                   num_devices=N_CORES)

    # xt: x.T in 16 K=64 chunks, DUPLICATED across partition halves so
    # both PE row-tiles can stream it.  weights: [128, 16, 512] with the
    # nt0 output columns on partitions 0:64 (tile T0) and nt1 on 64:128
    # (tile T8) -- each tile's 16-chunk chain yields a COMPLETE nt-half
    # in its own PSUM bank (no cross-tile partial-sum adds needed).
    xt_d = nc.declare_dram_parameter("xt", [PAIRS_PER_CORE, 128, 16, 128],
                                     dt.bfloat16, isOutput=False)
    w_d = {}
    for name in ("wq", "wk", "wv", "wo"):
        w_d[name] = nc.declare_dram_parameter(name + "t", [128, 16, 512],
                                              dt.bfloat16, isOutput=False)
    rc_d = nc.declare_dram_parameter("ropec", [128, 16, 32], dt.float32,
                                     isOutput=False)
    rs_d = nc.declare_dram_parameter("ropes", [128, 16, 32], dt.float32,
                                     isOutput=False)
    out_d = nc.declare_dram_parameter("out", [PAIRS_PER_CORE, 128, 1024],
                                      dt.float32, isOutput=True)

    with tile.TileContext(nc) as tc:
        with (
            tc.tile_pool(name="wts", bufs=1) as wts,
            tc.tile_pool(name="const", bufs=1) as constp,
            tc.tile_pool(name="xin", bufs=4) as xin,
            tc.tile_pool(name="tmp", bufs=4) as tmpp,
            tc.tile_pool(name="qkr", bufs=3) as qkrp,
            tc.tile_pool(name="vsb", bufs=1) as vsbp,
            tc.tile_pool(name="qkt", bufs=1) as qktp,
            tc.tile_pool(name="esb", bufs=8) as esbp,
            tc.tile_pool(name="rsb", bufs=2) as rsbp,
            tc.tile_pool(name="vtm", bufs=2) as vtmp_p,
            tc.tile_pool(name="osb", bufs=2) as osbp,
            tc.tile_pool(name="psA", bufs=1, space="PSUM") as psA,
            tc.tile_pool(name="psS", bufs=2, space="PSUM") as psS,
            tc.tile_pool(name="psW", bufs=2, space="PSUM") as psW,
        ):
            xs0 = xin.tile([128, 16, 128], dt.bfloat16, tag="xt")
            for hh in range(2):
                nc.sync.dma_start(xs0[:, 8 * hh:8 * hh + 8, :],
                                  xt_d[0][:, 8 * hh:8 * hh + 8, :])
            rc = constp.tile([128, 16, 32], dt.float32, tag="rc")
            rs = constp.tile([128, 16, 32], dt.float32, tag="rs")
            nc.scalar.dma_start(rc[:], rc_d[:])
            nc.scalar.dma_start(rs[:], rs_d[:])
            # touch Exp immediately so ACT_TABLE_LOAD runs during DMA warmup
            warm = constp.tile([1, 1], dt.float32, tag="warm")
            nc.scalar.activation(warm[:], rc[0:1, 0:1, 0:1], AF.Exp)
            ident = constp.tile([128, 128], dt.bfloat16, tag="id")
            make_identity(nc, ident[:])
            w_sb = {}
            dma_eng = {"wq": nc.sync, "wk": nc.scalar,
                       "wv": nc.sync, "wo": nc.scalar}
            for name in ("wq", "wk", "wv", "wo"):
                t = wts.tile([128, 16, 512], dt.bfloat16, tag=name)
                for kk in range(8):
                    dma_eng[name].dma_start(t[:, 2 * kk:2 * kk + 2, :],
                                            w_d[name][:, 2 * kk:2 * kk + 2, :])
                w_sb[name] = t

            # persistent q2t/k2t/vsb (double-buffered by pair parity).
            # q2t holds the 64 q channels DUPLICATED across partition
            # halves; k2t interleaves even chunks on 0:64, odd on 64:128;
            # vsb columns [ones | zeros(63) | V] (sums -> acc partition 0).
            q_tiles, k_tiles, v_tiles = [], [], []
            for vi in range(2):
                qt = qktp.tile([128, 16, 128], dt.bfloat16, tag=f"q2t{vi}",
                               name=f"q2t_{vi}")
                kt = qktp.tile([128, 8, 128], dt.bfloat16, tag=f"k2t{vi}",
                               name=f"k2t_{vi}")
                vt = vsbp.tile([128, 16, 128], dt.bfloat16, tag=f"v{vi}",
                               name=f"vsb{vi}")
                nc.gpsimd.memset(vt[:, :, 1:64], 0.0)
                nc.vector.memset(vt[:, :, 0:1], 1.0)
                q_tiles.append(qt)
                k_tiles.append(kt)
                v_tiles.append(vt)

            fin_work = []   # deferred output projection of the previous pair

            def fin_chain(wname, lhs, dst0, dst1):
                """16-chunk K=64 chains: T0 -> complete nt0 half in dst0,
                T8 -> nt1 in dst1 (lhs duplicated across partition halves)."""
                for c in range(16):
                    nc.tensor.matmul(
                        dst0, lhs[0:64, c, :], w_sb[wname][0:64, c, :],
                        start=(c == 0), stop=(c == 15))
                for c in range(16):
                    nc.tensor.matmul(
                        dst1, lhs[64:128, c, :], w_sb[wname][64:128, c, :],
                        start=(c == 0), stop=(c == 15))

            def emit_fin(otok_t, p_idx):
                # wo projection, contraction split T0/T8 -> finA/finB;
                # the A+B add replaces the old psum-evacuation copy.
                osb = osbp.tile([128, 1024], dt.float32, tag="osb")
                fin0 = psW.tile([128, 512], dt.float32, tag="w",
                                name=f"fin0_{p_idx}")
                fin1 = psW.tile([128, 512], dt.float32, tag="w",
                                name=f"fin1_{p_idx}")
                fin_chain("wo", otok_t, fin0[:], fin1[:])
                nc.vector.tensor_copy(osb[:, 0:512], fin0[:])
                nc.vector.tensor_copy(osb[:, 512:1024], fin1[:])
                nc.sync.dma_start(out_d[p_idx], osb[:])

            def rope_emit(pp, dst, sh):
                xe, xo = pp[:, :, 0, :], pp[:, :, 1, :]
                t1 = tmpp.tile([128, 8, 32], dt.float32, tag="t1")
                t2 = tmpp.tile([128, 8, 32], dt.float32, tag="t2")
                nc.vector.tensor_mul(t1[:], xe, rc[:, sh, :])
                nc.vector.tensor_mul(t2[:], xo, rs[:, sh, :])
                nc.vector.tensor_sub(dst[:, sh, 0, :], t1[:], t2[:])
                t3 = tmpp.tile([128, 8, 32], dt.float32, tag="t1")
                t4 = tmpp.tile([128, 8, 32], dt.float32, tag="t2")
                nc.vector.tensor_mul(t3[:], xe, rs[:, sh, :])
                nc.vector.tensor_mul(t4[:], xo, rc[:, sh, :])
                nc.vector.tensor_add(dst[:, sh, 1, :], t3[:], t4[:])

            def proj_transp(p):
                """projections + rope + transposes for pair p; returns
                (q2t, k2t, vsb) ready for attention."""
                if p == 0:
                    xs = xs0
                else:
                    xs = xin.tile([128, 16, 128], dt.bfloat16, tag="xt",
                                  name=f"xs{p}")
                    for hh in range(2):
                        nc.sync.dma_start(xs[:, 8 * hh:8 * hh + 8, :],
                                          xt_d[p][:, 8 * hh:8 * hh + 8, :])

                q2t = q_tiles[p % 2]
                k2t = k_tiles[p % 2]
                vsb = v_tiles[p % 2]

                qr = qkrp.tile([128, 16, 2, 32], dt.bfloat16, tag="qr",
                               name=f"qr{p}")
                kr = qkrp.tile([128, 16, 2, 32], dt.bfloat16, tag="kr",
                               name=f"kr{p}")

                def proj_qk(tname, dst):
                    pp0 = psW.tile([128, 8, 2, 32], dt.float32, tag="w",
                                   name=f"pp0{p}_{tname}")
                    pp1 = psW.tile([128, 8, 2, 32], dt.float32, tag="w",
                                   name=f"pp1{p}_{tname}")
                    fin_chain(tname, xs, pp0[:], pp1[:])
                    rope_emit(pp0[:], dst, slice(0, 8))
                    rope_emit(pp1[:], dst, slice(8, 16))

                if p == 0:
                    # prologue: all four chains (wq nt0/nt1 in psS, wk
                    # nt0/nt1 in psW) round-robined by 64-chunk so weight
                    # chunks are consumed on arrival
                    pq0 = psS.tile([128, 8, 2, 32], dt.float32, tag="s",
                                   name="pq0_0")
                    pq1 = psS.tile([128, 8, 2, 32], dt.float32, tag="s",
                                   name="pq0_1")
                    pk0 = psW.tile([128, 8, 2, 32], dt.float32, tag="w",
                                   name="pk0_0")
                    pk1 = psW.tile([128, 8, 2, 32], dt.float32, tag="w",
                                   name="pk0_1")
                    for c in range(16):
                        nc.tensor.matmul(
                            pq0[:], xs[0:64, c, :], w_sb["wq"][0:64, c, :],
                            start=(c == 0), stop=(c == 15))
                        nc.tensor.matmul(
                            pq1[:], xs[64:128, c, :],
                            w_sb["wq"][64:128, c, :],
                            start=(c == 0), stop=(c == 15))
                        nc.tensor.matmul(
                            pk0[:], xs[0:64, c, :], w_sb["wk"][0:64, c, :],
                            start=(c == 0), stop=(c == 15))
                        nc.tensor.matmul(
                            pk1[:], xs[64:128, c, :],
                            w_sb["wk"][64:128, c, :],
                            start=(c == 0), stop=(c == 15))
                    rope_emit(pq0[:], qr, slice(0, 8))
                    rope_emit(pq1[:], qr, slice(8, 16))
                    rope_emit(pk0[:], kr, slice(0, 8))
                    rope_emit(pk1[:], kr, slice(8, 16))
                else:
                    proj_qk("wq", qr)
                    proj_qk("wk", kr)

                # transposes as ROW-TILED matmuls against identity blocks:
                # T0 transposes tokens 0:63 -> tpA, T8 tokens 64:127 -> tpB
                # (adjacent psW slots = different banks).  Each handles an
                # s-PAIR: tp partitions 0:64 = s_even chans, 64:128 = odd.
                def transp_pair(src, j):
                    tpA = psW.tile([128, 64], dt.float32, tag="w",
                                   name=f"tpA{p}_{j}")
                    tpB = psW.tile([128, 64], dt.float32, tag="w",
                                   name=f"tpB{p}_{j}")
                    nc.tensor.matmul(
                        tpA[:], src[0:64, 2 * j:2 * j + 2, :, :],
                        ident[0:64, 0:64], start=True, stop=True)
                    nc.tensor.matmul(
                        tpB[:], src[64:128, 2 * j:2 * j + 2, :, :],
                        ident[64:128, 64:128], start=True, stop=True)
                    return tpA, tpB

                for j in range(8):
                    tpA, tpB = transp_pair(qr, j)
                    for sub in range(2):
                        s = 2 * j + sub
                        nc.vector.tensor_copy(
                            q2t[0:64, s, 0:64], tpA[sub * 64:(sub + 1) * 64, :])
                        nc.vector.tensor_copy(
                            q2t[0:64, s, 64:128],
                            tpB[sub * 64:(sub + 1) * 64, :])
                    if j == 3:
                        nc.sync.dma_start(q2t[64:128, 0:8, :],
                                          q2t[0:64, 0:8, :])
                nc.sync.dma_start(q2t[64:128, 8:16, :], q2t[0:64, 8:16, :])
                for j in range(8):
                    tpA, tpB = transp_pair(kr, j)
                    nc.vector.tensor_copy(k2t[:, j, 0:64], tpA[:])
                    nc.vector.tensor_copy(k2t[:, j, 64:128], tpB[:])

                # V projection: T0 -> complete nt0 half, T8 -> nt1
                v0 = psW.tile([128, 8, 64], dt.float32, tag="w",
                              name=f"v0_{p}")
                v1 = psW.tile([128, 8, 64], dt.float32, tag="w",
                              name=f"v1_{p}")
                fin_chain("wv", xs, v0[:], v1[:])
                nc.vector.tensor_copy(vsb[:, 0:8, 64:128], v0[:])
                nc.vector.tensor_copy(vsb[:, 8:16, 64:128], v1[:])
                return q2t, k2t, vsb

            def att_quarter(p, qh, tiles, otok, mid=None, late=None):
                """one quarter = 4 s-blocks (512 q cols) x all 16 chunks."""
                q2t, k2t, vsb = tiles
                accA = psA.tile([128, 4, 128], dt.float32, tag="accA",
                                name=f"accA{p}_{qh}")
                accB = psA.tile([128, 4, 128], dt.float32, tag="accB",
                                name=f"accB{p}_{qh}")
                s0 = qh * 4

                def av(e, i):
                    for ci in range(2):
                        c = 2 * i + ci
                        cs = slice(ci * 512, (ci + 1) * 512)
                        nc.tensor.matmul(
                            accA[:], vsb[0:64, c, :], e[0:64, cs],
                            start=(c == 0), stop=(c == 15),
                            skip_group_check=True)
                        nc.tensor.matmul(
                            accB[:], vsb[64:128, c, :], e[64:128, cs],
                            start=(c == 0), stop=(c == 15),
                            skip_group_check=True)

                pend = []
                for i in range(8):      # chunk-pair index
                    sct = psS.tile([128, 1024], dt.float32, tag="s",
                                   name=f"sct{p}_{qh}_{i}")
                    # row-tiled scores: even chunk on T0 -> cols 0:512
                    # (bank n), odd chunk on T8 -> cols 512:1024 (bank n+1)
                    nc.tensor.matmul(
                        sct[:, 0:512],
                        k2t[0:64, i, :],
                        q2t[0:64, s0:s0 + 4, :],
                        start=True, stop=True)
                    nc.tensor.matmul(
                        sct[:, 512:1024],
                        k2t[64:128, i, :],
                        q2t[64:128, s0:s0 + 4, :],
                        start=True, stop=True)
                    e = esbp.tile([128, 1024], dt.bfloat16, tag="e")
                    nc.scalar.activation(e[:], sct[:], AF.Exp, scale=0.125)
                    pend.append((e, i))
                    if len(pend) > 2:
                        av(*pend.pop(0))
                    if i == 2 and mid is not None:
                        mid()
                    if i == 5 and late is not None:
                        late()
                while pend:
                    av(*pend.pop(0))

                # normalize: accA + accB (keys split) -- accB is evacuated
                # to SBUF first (DVE has one PSUM read port), and the V-row
                # sum lands on partitions 0:64 so the otok muls read both
                # SBUF operands from the same start partition (walrus
                # requires SBUF inputs of tensor_tensor to align; PSUM
                # operands are exempt).
                cB = vtmp_p.tile([128, 4, 128], dt.float32, tag="cB")
                nc.vector.tensor_copy(cB[:], accB[:])
                sums = rsbp.tile([1, 4, 128], dt.float32, tag="sum")
                nc.vector.tensor_add(sums[:], accA[0:1, :, :],
                                     cB[0:1, :, :])
                asumV = vtmp_p.tile([64, 4, 128], dt.float32, tag="vt")
                nc.vector.tensor_add(asumV[:], accA[64:128, :, :],
                                     cB[64:128, :, :])
                rsb = rsbp.tile([1, 4, 128], dt.float32, tag="r")
                nc.vector.reciprocal_approx_fast(out=rsb[:], in_=sums[:])
                rbc = rsbp.tile([64, 4, 128], dt.float32, tag="rbc")
                nc.gpsimd.partition_broadcast(rbc[:], rsb[:])
                # otok is [64 d-chans, 16 s-chunks, 128 tok] (chunk = s),
                # so the quarter's 4 s-blocks land in one contiguous mul
                nc.vector.tensor_mul(otok[0:64, qh * 4:qh * 4 + 4, :],
                                     asumV[:], rbc[:])

            # pair-level software pipeline: proj/transp of pair p+1 and
            # fin of pair p-1 are emitted inside pair p's 3rd quarter
            tiles = proj_transp(0)
            nxt_box = [None]
            for p in range(PAIRS_PER_CORE):
                otok = osbp.tile([128, 16, 128], dt.bfloat16, tag="otok",
                                 name=f"otok{p}")

                def mid(p=p):
                    if p + 1 < PAIRS_PER_CORE:
                        nxt_box[0] = proj_transp(p + 1)

                def late():
                    while fin_work:
                        emit_fin(*fin_work.pop(0))

                for qh in range(4):
                    att_quarter(p, qh, tiles, otok,
                                mid=mid if qh == 2 else None,
                                late=late if qh == 2 else None)
                # duplicate otok across partition halves for the row-tiled
                # wo chains (T8 reads partitions 64:128)
                nc.sync.dma_start(otok[64:128, :, :], otok[0:64, :, :])
                fin_work.append((otok, p))
                tiles = nxt_box[0]

            while fin_work:
                emit_fin(*fin_work.pop(0))

    nc.compile()
    return nc



def _get_nc():
    if "nc" not in _CACHE:
        _CACHE["nc"] = _build_nc()
    return _CACHE["nc"]


def _prep_inputs(x, wq, wk, wv, wo):
    perm = _chan_perm()
    ropec, ropes = _rope_tables()

    def wt(w):
        # [out_chan, dim] -> [128, 16 K=64-chunks, 512]: rows 0:64 carry
        # the nt0 output columns (PE tile T0), rows 64:128 carry nt1 (T8)
        wT = np.ascontiguousarray(w.T)        # [dim, out_chan]
        out = np.empty((128, 16, 512), BF16)
        for c in range(16):
            out[0:64, c, :] = wT[c * 64:(c + 1) * 64, 0:512]
            out[64:128, c, :] = wT[c * 64:(c + 1) * 64, 512:1024]
        return out

    wqt = wt(wq[perm, :])
    wkt = wt(wk[perm, :])
    wvt = wt(wv)
    wot = wt(wo)

    in_maps = []
    for core in range(N_CORES):
        # x.T in 16 K=64 chunks, duplicated across partition halves
        xts = np.empty((PAIRS_PER_CORE, 128, 16, 128), BF16)
        for pl in range(PAIRS_PER_CORE):
            pg = core * PAIRS_PER_CORE + pl
            b, h = pg // H, pg % H
            XT = np.ascontiguousarray(
                x[b, h * 128:(h + 1) * 128, :].T)   # [1024, 128 tok]
            xc = XT.reshape(16, 64, 128).transpose(1, 0, 2)  # [64, 16, 128]
            xts[pl][0:64] = xc
            xts[pl][64:128] = xc
        in_maps.append({
            "xt": xts,
            "wqt": wqt, "wkt": wkt, "wvt": wvt, "wot": wot,
            "ropec": ropec, "ropes": ropes,
        })
    return in_maps


def run_sharded(x, wq, wk, wv, wo, trace=False, **run_kwargs):
    """Build + run on 8 cores; returns (full_output, BassKernelResults)."""
    from concourse.bass_utils import run_bass_kernel_spmd

    nc = _get_nc()
    in_maps = _prep_inputs(np.asarray(x, np.float32), np.asarray(wq, np.float32),
                           np.asarray(wk, np.float32), np.asarray(wv, np.float32),
                           np.asarray(wo, np.float32))
    res = run_bass_kernel_spmd(nc, in_maps, core_ids=list(range(N_CORES)),
                               trace=trace, **run_kwargs)
    out = np.empty((B, N, DIM), np.float32)
    for core in range(N_CORES):
        o = np.asarray(res.results[core]["out"], np.float32)
        for pl in range(PAIRS_PER_CORE):
            pg = core * PAIRS_PER_CORE + pl
            b, h = pg // H, pg % H
            out[b, h * 128:(h + 1) * 128, :] = o[pl]
    return out, res


def kernel(x, wq, wk, wv, wo):
    out, _ = run_sharded(x, wq, wk, wv, wo, trace=False)
    return out


# revision 35
# speedup vs baseline: 1.2392x; 1.2392x over previous
"""Trainium2 Bass kernel for the raw-reshape RoPE attention problem.

Math structure (verified against the reference):
  The reference reshapes [B, N, H*D] -> [B, H, N, D] with a *raw* reshape
  (no transpose).  Viewing the projection [2048, 1024] as [32768, 64],
  head h covers rows [h*2048, (h+1)*2048) -- i.e. head h only sees input
  tokens [h*128, (h+1)*128), and the final output rows [h*128, (h+1)*128)
  depend only on head h.  So the 32 (b, h) pairs are fully independent:
  4 pairs per NeuronCore, no collectives.

  Within a head we use the row permutation j' = s*128 + t (orig j = t*16+s,
  s = weight block, t = token).  Softmax/attention are invariant to a
  consistent row permutation of q/k/v; it makes every on-device layout
  change a contiguous [64, 128] block copy.

  RoPE rotates pair (2i, 2i+1) of each 64-channel block by the angle of
  position j-1 (row j=0 unrotated).  We pre-permute wq/wk rows on the host
  so even channels land in [s*64, s*64+32) and odd in [s*64+32, s*64+64),
  making the on-device rotation pure contiguous-block arithmetic.  The
  permutation cancels inside the q.k dot products.

Per (b, h) pair on device (all matmuls bf16, fp32 accumulation):
  Qp/Kp/Vp = Xp @ Wt           (lhsT = x.T blocks, host-pretransposed)
  rope(Qp), rope(Kp) on DVE    -> bf16 [128 tok, 1024 chan]
  PE-transpose + [64,128] block copies -> q2T/k2T, zero-padded to K=128
  (full-array matmuls keep one PE tiling mode -- mode switches drain the
  array, ~250ns per switch, measured)
  for j-half, key-chunk c: scoresT = k2T_c.T @ q2T; E = exp(s/8) (ScalarE)
  out_accT += [1 | 0.. | V_c].T @ E  (row 0 accumulates softmax sums; AV
  runs three chunks behind exp so ScalarE stays saturated)
  normalize via custom-DVE approx reciprocal + GPSIMD partition_broadcast
  final = out_tok @ wo.T (deferred into the next pair's attention window).

v2 changes vs the 227us baseline: q2t/k2t/vsb are PERSISTENT tiles whose
zero regions are written once at startup (kills ~8us/pair of gpsimd
memsets that also contended for the DVE SBUF port), and the wo-psum
evacuation runs on VectorE instead of ScalarE (ScalarE is the exp-bound
engine).

Schedule: pair-level software pipeline -- proj/rope/transposes of pair p+1
and the deferred wo-projection of pair p-1 are emitted inside pair p's
second attention half, landing in the PE slack of the ScalarE-bound
exp stream.
"""

import numpy as np
import ml_dtypes

N_CORES = 8
B, N, DIM = 2, 2048, 1024
H, HD = 16, 64
PAIRS_PER_CORE = 4
BF16 = ml_dtypes.bfloat16

_CACHE = {}


def _rope_tables():
    """cos/sin tables [128 t, 16 s, 32 i]; (t=0, s=0) is the unrotated row."""
    inv = 1.0 / (10000.0 ** (np.arange(0, HD, 2, dtype=np.float32) / HD))
    pos = np.arange(128 * 16, dtype=np.float32).reshape(128, 16) - 1.0  # j-1
    ang = pos[:, :, None] * inv[None, None, :]          # [128, 16, 32]
    c = np.cos(ang).astype(np.float32)
    s = np.sin(ang).astype(np.float32)
    c[0, 0, :] = 1.0
    s[0, 0, :] = 0.0
    return c, s


def _chan_perm():
    """c = s*64 + 2i + par -> c' = s*64 + par*32 + i."""
    perm = np.zeros(DIM, np.int64)
    for s in range(16):
        i = np.arange(32)
        perm[s * 64 + i] = s * 64 + 2 * i
        perm[s * 64 + 32 + i] = s * 64 + 2 * i + 1
    return perm


def _build_nc():
    import concourse.mybir as mybir
    import concourse.tile as tile
    from concourse import bacc
    from concourse.masks import make_identity

    dt = mybir.dt
    AF = mybir.ActivationFunctionType

    nc = bacc.Bacc("TRN2", target_bir_lowering=False, debug=False,
                   num_devices=N_CORES)

    xt_d = nc.declare_dram_parameter("xt", [PAIRS_PER_CORE, 128, 8, 128],
                                     dt.bfloat16, isOutput=False)
    w_d = {}
    for name in ("wq", "wk", "wv", "wo"):
        w_d[name] = nc.declare_dram_parameter(name + "t", [128, 8, 1024],
                                              dt.bfloat16, isOutput=False)
    rc_d = nc.declare_dram_parameter("ropec", [128, 16, 32], dt.float32,
                                     isOutput=False)
    rs_d = nc.declare_dram_parameter("ropes", [128, 16, 32], dt.float32,
                                     isOutput=False)
    out_d = nc.declare_dram_parameter("out", [PAIRS_PER_CORE, 128, 1024],
                                      dt.float32, isOutput=True)


    with tile.TileContext(nc) as tc:
        with (
            tc.tile_pool(name="wts", bufs=1) as wts,
            tc.tile_pool(name="const", bufs=1) as constp,
            tc.tile_pool(name="xin", bufs=4) as xin,
            tc.tile_pool(name="tmp", bufs=4) as tmpp,
            tc.tile_pool(name="qkr", bufs=3) as qkrp,
            tc.tile_pool(name="vsb", bufs=1) as vsbp,
            tc.tile_pool(name="qkt", bufs=1) as qktp,
            tc.tile_pool(name="esb", bufs=8) as esbp,
            tc.tile_pool(name="rsb", bufs=2) as rsbp,
            tc.tile_pool(name="osb", bufs=2) as osbp,
            tc.tile_pool(name="psA", bufs=1, space="PSUM") as psA,
            tc.tile_pool(name="psS", bufs=2, space="PSUM") as psS,
            tc.tile_pool(name="psW", bufs=2, space="PSUM") as psW,
        ):
            # pair-0 activations + rope tables first (unblock first proj),
            # then weights chunk-streamed across three DMA issue engines
            xs0 = xin.tile([128, 8, 128], dt.bfloat16, tag="xt")
            nc.sync.dma_start(xs0[:], xt_d[0])
            rc = constp.tile([128, 16, 32], dt.float32, tag="rc")
            rs = constp.tile([128, 16, 32], dt.float32, tag="rs")
            nc.scalar.dma_start(rc[:], rc_d[:])
            nc.scalar.dma_start(rs[:], rs_d[:])
            # touch Exp immediately so ACT_TABLE_LOAD runs during DMA warmup
            warm = constp.tile([1, 1], dt.float32, tag="warm")
            nc.scalar.activation(warm[:], rc[0:1, 0:1, 0:1], AF.Exp)
            ident = constp.tile([128, 128], dt.bfloat16, tag="id")
            make_identity(nc, ident[:])
            w_sb = {}
            # wq/wk first at full bandwidth; wv/wo queue behind them on
            # the same engines (pair-0 QK-proj is the critical path)
            dma_eng = {"wq": nc.sync, "wk": nc.scalar,
                       "wv": nc.sync, "wo": nc.scalar}
            for name in ("wq", "wk", "wv", "wo"):
                t = wts.tile([128, 8, 1024], dt.bfloat16, tag=name)
                for kk in range(8):
                    dma_eng[name].dma_start(t[:, kk, :], w_d[name][:, kk, :])
                w_sb[name] = t

            # persistent double-buffered k2t/vsb: the zero regions (PE pad
            # half, V zero block) and the softmax ones column are written
            # ONCE here; per-pair code only writes the data regions.  vsb
            # columns: [ones | zeros(63) | V(64)] so softmax sums land on
            # acc partition 0.  q lives in PAIR-INTERLEAVED tiles (q2i:
            # s-pairs (2j, 2j+1); q2off: offset pairs (2j+1, 2j+2))
            # produced by the DMA transpose XBAR -- the "wrong" s-block on
            # partitions 64:128 of each rhs is annihilated by k2t's zero
            # upper half in the scores matmul.
            qi_tiles, qo_tiles, k_tiles, v_tiles = [], [], [], []
            for vi in range(2):
                qi = qktp.tile([128, 8, 128], dt.bfloat16, tag=f"q2i{vi}",
                               name=f"q2i_{vi}")
                qo = qktp.tile([128, 8, 128], dt.bfloat16, tag=f"q2o{vi}",
                               name=f"q2o_{vi}")
                kt = qktp.tile([128, 16, 128], dt.bfloat16, tag=f"k2t{vi}",
                               name=f"k2t_{vi}")
                vt = vsbp.tile([128, 16, 128], dt.bfloat16, tag=f"v{vi}",
                               name=f"vsb{vi}")
                nc.gpsimd.memset(kt[64:128, :, :], 0.0)
                nc.gpsimd.memset(vt[:, :, 1:64], 0.0)
                nc.vector.memset(vt[:, :, 0:1], 1.0)
                qi_tiles.append(qi)
                qo_tiles.append(qo)
                k_tiles.append(kt)
                v_tiles.append(vt)

            fin_work = []   # deferred output projection of the previous pair

            def emit_fin(otok_t, p_idx):
                osb = osbp.tile([128, 1024], dt.float32, tag="osb")
                for nt in range(2):
                    fin = psW.tile([128, 512], dt.float32, tag="w")
                    for kk in range(8):
                        nc.tensor.matmul(
                            fin[:],
                            otok_t[:, kk, :],
                            w_sb["wo"][:, kk, nt * 512:(nt + 1) * 512],
                            start=(kk == 0), stop=(kk == 7))
                    nc.vector.tensor_copy(osb[:, nt * 512:(nt + 1) * 512],
                                          fin[:])
                nc.sync.dma_start(out_d[p_idx], osb[:])

            def proj_transp(p):
                """projections + rope + transposes for pair p; returns
                (q2t, k2t, vsb) ready for attention."""
                if p == 0:
                    xs = xs0
                else:
                    xs = xin.tile([128, 8, 128], dt.bfloat16, tag="xt",
                                  name=f"xs{p}")
                    nc.sync.dma_start(xs[:], xt_d[p])

                q2i = qi_tiles[p % 2]
                q2o = qo_tiles[p % 2]
                k2t = k_tiles[p % 2]
                vsb = v_tiles[p % 2]

                # qr has a zeroed 17th s-slot: the offset-pair transpose
                # (s15, pad) reads it; its output lands on partitions
                # 64:128 where k2t's zeros annihilate it (memset guards
                # against NaN garbage).
                qr = qkrp.tile([128, 17, 2, 32], dt.bfloat16, tag="qr",
                               name=f"qr{p}")
                nc.vector.memset(qr[:, 16, :, :], 0.0)
                kr = qkrp.tile([128, 16, 2, 32], dt.bfloat16, tag="kr",
                               name=f"kr{p}")

                ppool = psS if p == 0 else psW
                ptag = "s" if p == 0 else "w"

                def proj_qk(tname, dst):
                    for nt in range(2):
                        sh = slice(nt * 8, (nt + 1) * 8)
                        pp = ppool.tile([128, 8, 2, 32], dt.float32, tag=ptag)
                        for kk in range(8):
                            nc.tensor.matmul(
                                pp[:],
                                xs[:, kk, :],
                                w_sb[tname][:, kk, nt * 512:(nt + 1) * 512],
                                start=(kk == 0), stop=(kk == 7))
                        xe, xo = pp[:, :, 0, :], pp[:, :, 1, :]
                        t1 = tmpp.tile([128, 8, 32], dt.float32, tag="t1")
                        t2 = tmpp.tile([128, 8, 32], dt.float32, tag="t2")
                        nc.vector.tensor_mul(t1[:], xe, rc[:, sh, :])
                        nc.vector.tensor_mul(t2[:], xo, rs[:, sh, :])
                        nc.vector.tensor_sub(dst[:, sh, 0, :], t1[:], t2[:])
                        t3 = tmpp.tile([128, 8, 32], dt.float32, tag="t1")
                        t4 = tmpp.tile([128, 8, 32], dt.float32, tag="t2")
                        nc.vector.tensor_mul(t3[:], xe, rs[:, sh, :])
                        nc.vector.tensor_mul(t4[:], xo, rc[:, sh, :])
                        nc.vector.tensor_add(dst[:, sh, 1, :], t3[:], t4[:])

                if p == 0:
                    # prologue: round-robin the four Q/K accumulation chains
                    # by k-chunk so each weight chunk is consumed on arrival
                    pps = {}
                    for tname in ("wq", "wk"):
                        for nt in range(2):
                            pl, tg = (psS, "s") if tname == "wq" else (psW, "w")
                            pps[(tname, nt)] = pl.tile(
                                [128, 8, 2, 32], dt.float32, tag=tg,
                                name=f"pp0_{tname}_{nt}")
                    for kk in range(8):
                        for tname in ("wq", "wk"):
                            for nt in range(2):
                                nc.tensor.matmul(
                                    pps[(tname, nt)][:],
                                    xs[:, kk, :],
                                    w_sb[tname][:, kk,
                                                nt * 512:(nt + 1) * 512],
                                    start=(kk == 0), stop=(kk == 7))
                    for tname, dst in (("wq", qr), ("wk", kr)):
                        for nt in range(2):
                            sh = slice(nt * 8, (nt + 1) * 8)
                            pp = pps[(tname, nt)]
                            xe, xo = pp[:, :, 0, :], pp[:, :, 1, :]
                            t1 = tmpp.tile([128, 8, 32], dt.float32, tag="t1")
                            t2 = tmpp.tile([128, 8, 32], dt.float32, tag="t2")
                            nc.vector.tensor_mul(t1[:], xe, rc[:, sh, :])
                            nc.vector.tensor_mul(t2[:], xo, rs[:, sh, :])
                            nc.vector.tensor_sub(dst[:, sh, 0, :],
                                                 t1[:], t2[:])
                            t3 = tmpp.tile([128, 8, 32], dt.float32, tag="t1")
                            t4 = tmpp.tile([128, 8, 32], dt.float32, tag="t2")
                            nc.vector.tensor_mul(t3[:], xe, rs[:, sh, :])
                            nc.vector.tensor_mul(t4[:], xo, rc[:, sh, :])
                            nc.vector.tensor_add(dst[:, sh, 1, :],
                                                 t3[:], t4[:])
                else:
                    proj_qk("wq", qr)
                    proj_qk("wk", kr)

                # q transposes on the DMA XBAR (SBUF->SBUF, [128,128]
                # s-pair tiles); k on the PE with [64,128] DVE copies
                for j in range(8):
                    nc.sync.dma_start_transpose(
                        q2i[:, j, :], qr[:, 2 * j:2 * j + 2, :, :])
                for j in range(8):
                    nc.sync.dma_start_transpose(
                        q2o[:, j, :], qr[:, 2 * j + 1:2 * j + 3, :, :])
                for kk in range(8):
                    tp = psW.tile([128, 128], dt.bfloat16, tag="w")
                    nc.tensor.transpose(
                        tp[:], kr[:, 2 * kk:2 * kk + 2, :, :],
                        ident[:])
                    for sub in range(2):
                        s = 2 * kk + sub
                        nc.vector.tensor_copy(
                            k2t[0:64, s, :],
                            tp[sub * 64:(sub + 1) * 64, :])

                # V projection last: only needed once attention reaches AV.
                # Always psW -- pair 0's psS slots must be free for the
                # first scores
                for nt in range(2):
                    sh = slice(nt * 8, (nt + 1) * 8)
                    pp = psW.tile([128, 8, 64], dt.float32, tag="w")
                    for kk in range(8):
                        nc.tensor.matmul(
                            pp[:],
                            xs[:, kk, :],
                            w_sb["wv"][:, kk, nt * 512:(nt + 1) * 512],
                            start=(kk == 0), stop=(kk == 7))
                    nc.vector.tensor_copy(vsb[:, sh, 64:128], pp[:])
                return q2i, q2o, k2t, vsb

            def att_half(p, jh, tiles, otok, mid=None, late=None):
                q2i, q2o, k2t, vsb = tiles
                acc = psA.tile([128, 8, 128], dt.float32, tag="acc")

                def av(e, c):
                    for jt in range(2):
                        nc.tensor.matmul(
                            acc[:, jt * 4:(jt + 1) * 4, :],
                            vsb[:, c, :],
                            e[:, jt * 512:(jt + 1) * 512],
                            start=(c == 0), stop=(c == 15),
                            skip_group_check=True)

                pend = []
                for c in range(16):
                    e = esbp.tile([128, 1024], dt.bfloat16, tag="e")
                    sct = psS.tile([128, 1024], dt.float32, tag="s")
                    # col block 0 = the half's EVEN s-blocks (from q2i),
                    # block 1 = ODD (from q2off); k2t's zero upper half
                    # annihilates the other s-block on partitions 64:128
                    for jt, qsrc in ((0, q2i), (1, q2o)):
                        nc.tensor.matmul(
                            sct[:, jt * 512:(jt + 1) * 512],
                            k2t[:, c, :],
                            qsrc[:, jh * 4:jh * 4 + 4, :],
                            start=True, stop=True)
                    nc.scalar.activation(e[:], sct[:], AF.Exp, scale=0.125)
                    # AV runs two chunks behind so scores/exp stay ahead
                    pend.append((e, c))
                    if len(pend) > 3:
                        av(*pend.pop(0))
                    if c == 4 and mid is not None:
                        mid()
                    if c == 8 and late is not None:
                        late()
                while pend:
                    av(*pend.pop(0))

                # normalize + assemble out_tok.T
                rsb = rsbp.tile([1, 8, 128], dt.float32, tag="r")
                nc.vector.reciprocal_approx_fast(out=rsb[:], in_=acc[0:1, :, :])
                rbc = rsbp.tile([64, 8, 128], dt.float32, tag="rbc")
                nc.gpsimd.partition_broadcast(rbc[:], rsb[:])
                # acc cols 0:4 = the half's even s-blocks (par 0 -> otok
                # rows 0:64), cols 4:8 = odd (par 1 -> rows 64:128);
                # otok free idx g = s//2 = jh*4 + m in both blocks
                for par in range(2):
                    nc.vector.tensor_mul(
                        otok[par * 64:par * 64 + 64,
                             jh * 4:(jh + 1) * 4, :],
                        acc[64:128, par * 4:(par + 1) * 4, :],
                        rbc[:, par * 4:(par + 1) * 4, :])

            # pair-level software pipeline: proj/transp of pair p+1 is
            # emitted between pair p's two attention halves, fin of p-1
            # right after it -- both land in attention's ACT-bound PE slack
            tiles = proj_transp(0)
            nxt_box = [None]
            for p in range(PAIRS_PER_CORE):
                otok = osbp.tile([128, 8, 128], dt.bfloat16, tag="otok",
                                 name=f"otok{p}")
                att_half(p, 0, tiles, otok)

                def mid(p=p):
                    if p + 1 < PAIRS_PER_CORE:
                        nxt_box[0] = proj_transp(p + 1)

                def late():
                    while fin_work:
                        emit_fin(*fin_work.pop(0))

                att_half(p, 1, tiles, otok, mid=mid, late=late)
                fin_work.append((otok, p))
                tiles = nxt_box[0]

            while fin_work:
                emit_fin(*fin_work.pop(0))

    nc.compile()
    return nc



def _get_nc():
    if "nc" not in _CACHE:
        _CACHE["nc"] = _build_nc()
    return _CACHE["nc"]


def _prep_inputs(x, wq, wk, wv, wo):
    perm = _chan_perm()
    ropec, ropes = _rope_tables()

    def wt(w):
        # [out_chan, dim] -> transposed, partition-major [128, 8, 1024]
        return np.ascontiguousarray(
            w.T.reshape(8, 128, DIM).transpose(1, 0, 2)).astype(BF16)

    wqt = wt(wq[perm, :])
    wkt = wt(wk[perm, :])
    wvt = wt(wv)
    wot = wt(wo)

    in_maps = []
    for core in range(N_CORES):
        xts = np.empty((PAIRS_PER_CORE, 128, 8, 128), BF16)
        for pl in range(PAIRS_PER_CORE):
            pg = core * PAIRS_PER_CORE + pl
            b, h = pg // H, pg % H
            X = x[b, h * 128:(h + 1) * 128, :]      # [128 tok, 1024]
            xts[pl] = np.ascontiguousarray(
                X.T.reshape(8, 128, 128).transpose(1, 0, 2)).astype(BF16)
        in_maps.append({
            "xt": xts,
            "wqt": wqt, "wkt": wkt, "wvt": wvt, "wot": wot,
            "ropec": ropec, "ropes": ropes,
        })
    return in_maps


def run_sharded(x, wq, wk, wv, wo, trace=False, **run_kwargs):
    """Build + run on 8 cores; returns (full_output, BassKernelResults)."""
    from concourse.bass_utils import run_bass_kernel_spmd

    nc = _get_nc()
    in_maps = _prep_inputs(np.asarray(x, np.float32), np.asarray(wq, np.float32),
                           np.asarray(wk, np.float32), np.asarray(wv, np.float32),
                           np.asarray(wo, np.float32))
    res = run_bass_kernel_spmd(nc, in_maps, core_ids=list(range(N_CORES)),
                               trace=trace, **run_kwargs)
    out = np.empty((B, N, DIM), np.float32)
    for core in range(N_CORES):
        o = np.asarray(res.results[core]["out"], np.float32)
        for pl in range(PAIRS_PER_CORE):
            pg = core * PAIRS_PER_CORE + pl
            b, h = pg // H, pg % H
            out[b, h * 128:(h + 1) * 128, :] = o[pl]
    return out, res


def kernel(x, wq, wk, wv, wo):
    out, _ = run_sharded(x, wq, wk, wv, wo, trace=False)
    return out


# revision 37
# speedup vs baseline: 1.5833x; 1.2777x over previous
"""Trainium2 Bass kernel for the raw-reshape RoPE attention problem.

Math structure (verified against the reference):
  The reference reshapes [B, N, H*D] -> [B, H, N, D] with a *raw* reshape
  (no transpose).  Viewing the projection [2048, 1024] as [32768, 64],
  head h covers rows [h*2048, (h+1)*2048) -- i.e. head h only sees input
  tokens [h*128, (h+1)*128), and the final output rows [h*128, (h+1)*128)
  depend only on head h.  So the 32 (b, h) pairs are fully independent:
  4 pairs per NeuronCore, no collectives.

  Within a head we use the row permutation j' = s*128 + t (orig j = t*16+s,
  s = weight block, t = token).  Softmax/attention are invariant to a
  consistent row permutation of q/k/v; it makes every on-device layout
  change a contiguous [64, 128] block copy.

  RoPE rotates pair (2i, 2i+1) of each 64-channel block by the angle of
  position j-1 (row j=0 unrotated).  We pre-permute wq/wk rows on the host
  so even channels land in [s*64, s*64+32) and odd in [s*64+32, s*64+64),
  making the on-device rotation pure contiguous-block arithmetic.  The
  permutation cancels inside the q.k dot products.

Per (b, h) pair on device (all matmuls bf16, fp32 accumulation):
  Qp/Kp/Vp = Xp @ Wt           (lhsT = x.T blocks, host-pretransposed)
  rope(Qp), rope(Kp) on DVE    -> bf16 [128 tok, 1024 chan]
  PE-transpose + [64,128] block copies -> q2T/k2T, zero-padded to K=128
  (full-array matmuls keep one PE tiling mode -- mode switches drain the
  array, ~250ns per switch, measured)
  for j-half, key-chunk c: scoresT = k2T_c.T @ q2T; E = exp(s/8) (ScalarE)
  out_accT += [1 | 0.. | V_c].T @ E  (row 0 accumulates softmax sums; AV
  runs three chunks behind exp so ScalarE stays saturated)
  normalize via custom-DVE approx reciprocal + GPSIMD partition_broadcast
  final = out_tok @ wo.T (deferred into the next pair's attention window).

v2 changes vs the 227us baseline: q2t/k2t/vsb are PERSISTENT tiles whose
zero regions are written once at startup (kills ~8us/pair of gpsimd
memsets that also contended for the DVE SBUF port), and the wo-psum
evacuation runs on VectorE instead of ScalarE (ScalarE is the exp-bound
engine).

Schedule: pair-level software pipeline -- proj/rope/transposes of pair p+1
and the deferred wo-projection of pair p-1 are emitted inside pair p's
second attention half, landing in the PE slack of the ScalarE-bound
exp stream.
"""

import numpy as np
import ml_dtypes

N_CORES = 8
B, N, DIM = 2, 2048, 1024
H, HD = 16, 64
PAIRS_PER_CORE = 4
BF16 = ml_dtypes.bfloat16

_CACHE = {}


def _rope_tables():
    """cos/sin tables [128 t, 16 s, 32 i]; (t=0, s=0) is the unrotated row."""
    inv = 1.0 / (10000.0 ** (np.arange(0, HD, 2, dtype=np.float32) / HD))
    pos = np.arange(128 * 16, dtype=np.float32).reshape(128, 16) - 1.0  # j-1
    ang = pos[:, :, None] * inv[None, None, :]          # [128, 16, 32]
    c = np.cos(ang).astype(np.float32)
    s = np.sin(ang).astype(np.float32)
    c[0, 0, :] = 1.0
    s[0, 0, :] = 0.0
    return c, s


def _chan_perm():
    """c = s*64 + 2i + par -> c' = s*64 + par*32 + i."""
    perm = np.zeros(DIM, np.int64)
    for s in range(16):
        i = np.arange(32)
        perm[s * 64 + i] = s * 64 + 2 * i
        perm[s * 64 + 32 + i] = s * 64 + 2 * i + 1
    return perm


def _build_nc():
    import concourse.mybir as mybir
    import concourse.tile as tile
    from concourse import bacc
    from concourse.masks import make_identity

    dt = mybir.dt
    AF = mybir.ActivationFunctionType

    nc = bacc.Bacc("TRN2", target_bir_lowering=False, debug=False,
                   num_devices=N_CORES)

    xt_d = nc.declare_dram_parameter("xt", [PAIRS_PER_CORE, 128, 8, 128],
                                     dt.bfloat16, isOutput=False)
    w_d = {}
    for name in ("wq", "wk", "wv", "wo"):
        w_d[name] = nc.declare_dram_parameter(name + "t", [128, 8, 1024],
                                              dt.bfloat16, isOutput=False)
    rc_d = nc.declare_dram_parameter("ropec", [128, 16, 32], dt.float32,
                                     isOutput=False)
    rs_d = nc.declare_dram_parameter("ropes", [128, 16, 32], dt.float32,
                                     isOutput=False)
    out_d = nc.declare_dram_parameter("out", [PAIRS_PER_CORE, 128, 1024],
                                      dt.float32, isOutput=True)


    with tile.TileContext(nc) as tc:
        with (
            tc.tile_pool(name="wts", bufs=1) as wts,
            tc.tile_pool(name="const", bufs=1) as constp,
            tc.tile_pool(name="xin", bufs=4) as xin,
            tc.tile_pool(name="tmp", bufs=4) as tmpp,
            tc.tile_pool(name="qkr", bufs=3) as qkrp,
            tc.tile_pool(name="vsb", bufs=1) as vsbp,
            tc.tile_pool(name="qkt", bufs=1) as qktp,
            tc.tile_pool(name="esb", bufs=8) as esbp,
            tc.tile_pool(name="rsb", bufs=2) as rsbp,
            tc.tile_pool(name="osb", bufs=2) as osbp,
            tc.tile_pool(name="psA", bufs=1, space="PSUM") as psA,
            tc.tile_pool(name="psS", bufs=2, space="PSUM") as psS,
            tc.tile_pool(name="psW", bufs=2, space="PSUM") as psW,
        ):
            # pair-0 activations + rope tables first (unblock first proj),
            # then weights chunk-streamed across three DMA issue engines.
            # xs0 and the FIRST wq/wk chunks are split across several
            # dma_starts (one dma_start = one ~22GB/s hardware queue, so a
            # 256KB transfer alone takes ~11.6us) -- the first projection
            # matmul can then start at ~6us instead of ~12us.
            xs0 = xin.tile([128, 8, 128], dt.bfloat16, tag="xt")
            for qq in range(4):
                nc.sync.dma_start(xs0[:, 2 * qq:2 * qq + 2, :],
                                  xt_d[0][:, 2 * qq:2 * qq + 2, :])
            rc = constp.tile([128, 16, 32], dt.float32, tag="rc")
            rs = constp.tile([128, 16, 32], dt.float32, tag="rs")
            nc.scalar.dma_start(rc[:], rc_d[:])
            nc.scalar.dma_start(rs[:], rs_d[:])
            # touch Exp immediately so ACT_TABLE_LOAD runs during DMA warmup
            warm = constp.tile([1, 1], dt.float32, tag="warm")
            nc.scalar.activation(warm[:], rc[0:1, 0:1, 0:1], AF.Exp)
            ident = constp.tile([128, 128], dt.bfloat16, tag="id")
            make_identity(nc, ident[:])
            w_sb = {}
            # wq/wk first at full bandwidth; wv/wo queue behind them on
            # the same engines (pair-0 QK-proj is the critical path)
            dma_eng = {"wq": nc.sync, "wk": nc.scalar,
                       "wv": nc.sync, "wo": nc.scalar}
            for name in ("wq", "wk", "wv", "wo"):
                t = wts.tile([128, 8, 1024], dt.bfloat16, tag=name)
                for kk in range(8):
                    if kk == 0 and name in ("wq", "wk"):
                        for hh in range(2):
                            cs = slice(512 * hh, 512 * hh + 512)
                            dma_eng[name].dma_start(t[:, 0, cs],
                                                    w_d[name][:, 0, cs])
                    else:
                        dma_eng[name].dma_start(t[:, kk, :],
                                                w_d[name][:, kk, :])
                w_sb[name] = t

            # PE warm-up: the HAM clock gate holds the PE at 1.2GHz until
            # ~3.4us of sustained activity.  These identity matmuls depend
            # only on on-device data, so they run during the weight-DMA
            # window and the real pair-0 projections start at 2.4GHz.
            for wu in range(35):
                scr = psW.tile([128, 128], dt.float32, tag="w",
                               name=f"warmmm{wu}")
                nc.tensor.matmul(scr[:], ident[:], ident[:],
                                 start=True, stop=True)

            # persistent double-buffered q2t/k2t/vsb: the zero regions
            # (PE pad halves, V zero block) and the softmax ones column
            # are written ONCE here; per-pair code only writes the data
            # regions.  vsb columns: [ones | zeros(63) | V(64)] so softmax
            # sums land on acc partition 0.
            q_tiles, k_tiles, v_tiles = [], [], []
            for vi in range(2):
                qt = qktp.tile([128, 16, 128], dt.bfloat16, tag=f"q2t{vi}",
                               name=f"q2t_{vi}")
                kt = qktp.tile([128, 16, 128], dt.bfloat16, tag=f"k2t{vi}",
                               name=f"k2t_{vi}")
                vt = vsbp.tile([128, 16, 128], dt.bfloat16, tag=f"v{vi}",
                               name=f"vsb{vi}")
                nc.gpsimd.memset(qt[64:128, :, :], 0.0)
                nc.gpsimd.memset(kt[64:128, :, :], 0.0)
                nc.gpsimd.memset(vt[:, :, 1:64], 0.0)
                nc.vector.memset(vt[:, :, 0:1], 1.0)
                q_tiles.append(qt)
                k_tiles.append(kt)
                v_tiles.append(vt)

            fin_work = []   # deferred output projection of the previous pair

            def emit_fin(otok_t, p_idx):
                osb = osbp.tile([128, 1024], dt.float32, tag="osb")
                for nt in range(2):
                    fin = psW.tile([128, 512], dt.float32, tag="w")
                    for kk in range(8):
                        nc.tensor.matmul(
                            fin[:],
                            otok_t[:, kk, :],
                            w_sb["wo"][:, kk, nt * 512:(nt + 1) * 512],
                            start=(kk == 0), stop=(kk == 7))
                    nc.vector.tensor_copy(osb[:, nt * 512:(nt + 1) * 512],
                                          fin[:])
                nc.sync.dma_start(out_d[p_idx], osb[:])

            def proj_transp(p):
                """projections + rope + transposes for pair p; returns
                (q2t, k2t, vsb) ready for attention."""
                if p == 0:
                    xs = xs0
                else:
                    xs = xin.tile([128, 8, 128], dt.bfloat16, tag="xt",
                                  name=f"xs{p}")
                    nc.sync.dma_start(xs[:], xt_d[p])

                q2t = q_tiles[p % 2]
                k2t = k_tiles[p % 2]
                vsb = v_tiles[p % 2]

                qr = qkrp.tile([128, 16, 2, 32], dt.bfloat16, tag="qr",
                               name=f"qr{p}")
                kr = qkrp.tile([128, 16, 2, 32], dt.bfloat16, tag="kr",
                               name=f"kr{p}")

                ppool = psS if p == 0 else psW
                ptag = "s" if p == 0 else "w"

                def proj_qk(tname, dst):
                    for nt in range(2):
                        sh = slice(nt * 8, (nt + 1) * 8)
                        pp = ppool.tile([128, 8, 2, 32], dt.float32, tag=ptag)
                        for kk in range(8):
                            nc.tensor.matmul(
                                pp[:],
                                xs[:, kk, :],
                                w_sb[tname][:, kk, nt * 512:(nt + 1) * 512],
                                start=(kk == 0), stop=(kk == 7))
                        xe, xo = pp[:, :, 0, :], pp[:, :, 1, :]
                        t1 = tmpp.tile([128, 8, 32], dt.float32, tag="t1")
                        t2 = tmpp.tile([128, 8, 32], dt.float32, tag="t2")
                        nc.vector.tensor_mul(t1[:], xe, rc[:, sh, :])
                        nc.vector.tensor_mul(t2[:], xo, rs[:, sh, :])
                        nc.vector.tensor_sub(dst[:, sh, 0, :], t1[:], t2[:])
                        t3 = tmpp.tile([128, 8, 32], dt.float32, tag="t1")
                        t4 = tmpp.tile([128, 8, 32], dt.float32, tag="t2")
                        nc.vector.tensor_mul(t3[:], xe, rs[:, sh, :])
                        nc.vector.tensor_mul(t4[:], xo, rc[:, sh, :])
                        nc.vector.tensor_add(dst[:, sh, 1, :], t3[:], t4[:])

                if p == 0:
                    # prologue: round-robin the four Q/K accumulation chains
                    # by k-chunk so each weight chunk is consumed on arrival
                    pps = {}
                    for tname in ("wq", "wk"):
                        for nt in range(2):
                            pl, tg = (psS, "s") if tname == "wq" else (psW, "w")
                            pps[(tname, nt)] = pl.tile(
                                [128, 8, 2, 32], dt.float32, tag=tg,
                                name=f"pp0_{tname}_{nt}")
                    for kk in range(8):
                        for tname in ("wq", "wk"):
                            for nt in range(2):
                                nc.tensor.matmul(
                                    pps[(tname, nt)][:],
                                    xs[:, kk, :],
                                    w_sb[tname][:, kk,
                                                nt * 512:(nt + 1) * 512],
                                    start=(kk == 0), stop=(kk == 7))
                    for tname, dst in (("wq", qr), ("wk", kr)):
                        for nt in range(2):
                            sh = slice(nt * 8, (nt + 1) * 8)
                            pp = pps[(tname, nt)]
                            xe, xo = pp[:, :, 0, :], pp[:, :, 1, :]
                            t1 = tmpp.tile([128, 8, 32], dt.float32, tag="t1")
                            t2 = tmpp.tile([128, 8, 32], dt.float32, tag="t2")
                            nc.vector.tensor_mul(t1[:], xe, rc[:, sh, :])
                            nc.vector.tensor_mul(t2[:], xo, rs[:, sh, :])
                            nc.vector.tensor_sub(dst[:, sh, 0, :],
                                                 t1[:], t2[:])
                            t3 = tmpp.tile([128, 8, 32], dt.float32, tag="t1")
                            t4 = tmpp.tile([128, 8, 32], dt.float32, tag="t2")
                            nc.vector.tensor_mul(t3[:], xe, rs[:, sh, :])
                            nc.vector.tensor_mul(t4[:], xo, rc[:, sh, :])
                            nc.vector.tensor_add(dst[:, sh, 1, :],
                                                 t3[:], t4[:])
                else:
                    proj_qk("wq", qr)
                    proj_qk("wk", kr)

                # transposes on the PE; [64, 128] block copies on DVE
                for kk in range(8):
                    for src_, dstT in ((qr, q2t), (kr, k2t)):
                        tp = psW.tile([128, 128], dt.bfloat16, tag="w")
                        nc.tensor.transpose(
                            tp[:], src_[:, 2 * kk:2 * kk + 2, :, :],
                            ident[:])
                        for sub in range(2):
                            s = 2 * kk + sub
                            nc.vector.tensor_copy(
                                dstT[0:64, s, :],
                                tp[sub * 64:(sub + 1) * 64, :])

                # V projection last: only needed once attention reaches AV.
                # Always psW -- pair 0's psS slots must be free for the
                # first scores
                for nt in range(2):
                    sh = slice(nt * 8, (nt + 1) * 8)
                    pp = psW.tile([128, 8, 64], dt.float32, tag="w")
                    for kk in range(8):
                        nc.tensor.matmul(
                            pp[:],
                            xs[:, kk, :],
                            w_sb["wv"][:, kk, nt * 512:(nt + 1) * 512],
                            start=(kk == 0), stop=(kk == 7))
                    nc.vector.tensor_copy(vsb[:, sh, 64:128], pp[:])
                return q2t, k2t, vsb

            def att_half(p, jh, tiles, otok, mid=None, late=None):
                q2t, k2t, vsb = tiles
                acc = psA.tile([128, 8, 128], dt.float32, tag="acc")

                def av(e, c):
                    for jt in range(2):
                        nc.tensor.matmul(
                            acc[:, jt * 4:(jt + 1) * 4, :],
                            vsb[:, c, :],
                            e[:, jt * 512:(jt + 1) * 512],
                            start=(c == 0), stop=(c == 15),
                            skip_group_check=True)

                pend = []
                for c in range(16):
                    e = esbp.tile([128, 1024], dt.bfloat16, tag="e")
                    sct = psS.tile([128, 1024], dt.float32, tag="s")
                    for jt in range(2):
                        s0 = jh * 8 + jt * 4
                        nc.tensor.matmul(
                            sct[:, jt * 512:(jt + 1) * 512],
                            k2t[:, c, :],
                            q2t[:, s0:s0 + 4, :],
                            start=True, stop=True)
                    nc.scalar.activation(e[:], sct[:], AF.Exp, scale=0.125)
                    # AV runs two chunks behind so scores/exp stay ahead
                    pend.append((e, c))
                    if len(pend) > 3:
                        av(*pend.pop(0))
                    if c == 4 and mid is not None:
                        mid()
                    if c == 8 and late is not None:
                        late()
                while pend:
                    av(*pend.pop(0))

                # normalize + assemble out_tok.T
                rsb = rsbp.tile([1, 8, 128], dt.float32, tag="r")
                nc.vector.reciprocal_approx_fast(out=rsb[:], in_=acc[0:1, :, :])
                rbc = rsbp.tile([64, 8, 128], dt.float32, tag="rbc")
                nc.gpsimd.partition_broadcast(rbc[:], rsb[:])
                # two strided muls: even s-blocks -> otok rows 0:64,
                # odd -> rows 64:128 (s = jh*8 + sl; otok free idx s//2)
                for par in range(2):
                    nc.vector.tensor_mul(
                        otok[par * 64:par * 64 + 64,
                             jh * 4:(jh + 1) * 4, :],
                        acc[64:128, par:8:2, :],
                        rbc[:, par:8:2, :])

            # pair-level software pipeline: proj/transp of pair p+1 is
            # emitted between pair p's two attention halves, fin of p-1
            # right after it -- both land in attention's ACT-bound PE slack
            tiles = proj_transp(0)
            nxt_box = [None]
            for p in range(PAIRS_PER_CORE):
                otok = osbp.tile([128, 8, 128], dt.bfloat16, tag="otok",
                                 name=f"otok{p}")
                att_half(p, 0, tiles, otok)

                def mid(p=p):
                    if p + 1 < PAIRS_PER_CORE:
                        nxt_box[0] = proj_transp(p + 1)

                def late():
                    while fin_work:
                        emit_fin(*fin_work.pop(0))

                att_half(p, 1, tiles, otok, mid=mid, late=late)
                fin_work.append((otok, p))
                tiles = nxt_box[0]

            while fin_work:
                emit_fin(*fin_work.pop(0))

    nc.compile()
    return nc



def _get_nc():
    if "nc" not in _CACHE:
        _CACHE["nc"] = _build_nc()
    return _CACHE["nc"]


def _prep_inputs(x, wq, wk, wv, wo):
    perm = _chan_perm()
    ropec, ropes = _rope_tables()

    def wt(w):
        # [out_chan, dim] -> transposed, partition-major [128, 8, 1024]
        return np.ascontiguousarray(
            w.T.reshape(8, 128, DIM).transpose(1, 0, 2)).astype(BF16)

    wqt = wt(wq[perm, :])
    wkt = wt(wk[perm, :])
    wvt = wt(wv)
    wot = wt(wo)

    in_maps = []
    for core in range(N_CORES):
        xts = np.empty((PAIRS_PER_CORE, 128, 8, 128), BF16)
        for pl in range(PAIRS_PER_CORE):
            pg = core * PAIRS_PER_CORE + pl
            b, h = pg // H, pg % H
            X = x[b, h * 128:(h + 1) * 128, :]      # [128 tok, 1024]
            xts[pl] = np.ascontiguousarray(
                X.T.reshape(8, 128, 128).transpose(1, 0, 2)).astype(BF16)
        in_maps.append({
            "xt": xts,
            "wqt": wqt, "wkt": wkt, "wvt": wvt, "wot": wot,
            "ropec": ropec, "ropes": ropes,
        })
    return in_maps


def run_sharded(x, wq, wk, wv, wo, trace=False, **run_kwargs):
    """Build + run on 8 cores; returns (full_output, BassKernelResults)."""
    from concourse.bass_utils import run_bass_kernel_spmd

    nc = _get_nc()
    in_maps = _prep_inputs(np.asarray(x, np.float32), np.asarray(wq, np.float32),
                           np.asarray(wk, np.float32), np.asarray(wv, np.float32),
                           np.asarray(wo, np.float32))
    res = run_bass_kernel_spmd(nc, in_maps, core_ids=list(range(N_CORES)),
                               trace=trace, **run_kwargs)
    out = np.empty((B, N, DIM), np.float32)
    for core in range(N_CORES):
        o = np.asarray(res.results[core]["out"], np.float32)
        for pl in range(PAIRS_PER_CORE):
            pg = core * PAIRS_PER_CORE + pl
            b, h = pg // H, pg % H
            out[b, h * 128:(h + 1) * 128, :] = o[pl]
    return out, res


def kernel(x, wq, wk, wv, wo):
    out, _ = run_sharded(x, wq, wk, wv, wo, trace=False)
    return out
